# revision 6
# baseline (speedup 1.0000x reference)
"""Trainium2 Bass kernel for nn_PointerNetwork (B=512, L=256, H=512, F=1).

Strategy (8 NeuronCores, data-parallel over batch, 64 rows/core):
  - F=1 lets every input-side matmul fold to a rank-1 outer product, so the
    per-step LSTM work is just Whh @ h (true-fp32 PE matmuls for accuracy).
  - Gates use sigmoid(x) = 0.5*(1+tanh(x/2)) via the ACT tanh table (the
    native sigmoid table is ~10x less accurate); state is kept doubled
    (H = 2h, C = 2c) with the 0.5 folded into pre-scaled weights host-side.
  - Encoder also emits enc_projT = 0.5*W1 @ H + b incrementally, staged to an
    internal HBM tensor in [hc][128, (b,l)] layout (33.5 MB/core).
  - Decode loop (sequential, 256 steps): stream enc_projT from HBM,
    DVE tensor_scalar adds q (per-partition scalar) in place, ACT tanh in
    place, then a true-fp32 PE pass contracts with v using a sliding-window
    block-diagonal weight trick, giving u[64, 256] directly in PSUM.
    Softmax pieces: DVE max8/max_index (argmax), ACT exp with accumulate.
    log(S) is deferred to an end phase (ln lives in a different ACT table).
  - Numerics are fp32-class end to end (validated: argmax trajectory matches
    a float64 reference exactly; top-2 gaps as small as 3.7e-7 demand this).

kernel(**inputs) -> (attentions [512,256,256] f32, pointers [512,256] f32)
"""

import numpy as np

B, L, H, F = 512, 256, 512, 1
NCORES = 8
BL = B // NCORES            # 64
H4 = 4 * H                  # 2048
SOS = -1.0
KC = H // 128               # 4 k-chunks of the hidden dim
MG = H4 // 128              # 16 m-chunks of the gate dim
BGRP = 16                   # batch rows per streamed attention tile
NBG = BL // BGRP            # 4 tiles per hc chunk
LAND_W = BGRP * L           # 4096 free elems per land tile

_PROGRAM_CACHE = {}
LAST_EXEC_NS = None


def build_program(l_enc=L, l_dec=L, land_bufs=4):
    import concourse.bacc as bacc
    import concourse.bass as bass
    import concourse.tile as tile
    import concourse.mybir as mybir
    from contextlib import ExitStack

    fp32 = mybir.dt.float32
    AF = mybir.ActivationFunctionType
    OP = mybir.AluOpType
    ds = bass.ds

    nbg = NBG
    n_land = KC * nbg                      # 16 streamed tiles per decode step

    nc = bacc.Bacc("TRN2", target_bir_lowering=False, debug=False,
                   enable_asserts=True, num_devices=NCORES)

    # ---------------- DRAM I/O ----------------
    def din(name, shape):
        return nc.dram_tensor(name, shape, fp32, kind="ExternalInput").ap()

    xT_in = din("xT_in", [l_enc, BL])          # encoder inputs, transposed
    x_in = din("x_in", [BL, L])                # gather source (full L always)
    enc_aug = din("enc_aug", [2, H4])          # rows: [Wih@embW, Wih@embb + biases]
    dec_aug = din("dec_aug", [2, H4])
    WhhT_e = din("WhhT_e", [H, H4])            # (0.5*enc_Whh).T
    WhhT_d = din("WhhT_d", [H, H4])
    W1T = din("W1T", [H, H])                   # (0.5*W1_W).T
    W2T = din("W2T", [H, H])
    W1b4 = din("W1b4", [128, KC])              # W1_b chunks: [part, chunk]
    W2b4 = din("W2b4", [128, KC])
    vZ = din("vZ", [128, KC * 127])            # sliding-window v tiles
    ident = din("ident", [BL, BL])
    iota_f = din("iota_f", [BL, L])

    attn_out = nc.dram_tensor("attn_out", [BL, l_dec, L], fp32,
                              kind="ExternalOutput").ap()
    ptr_out = nc.dram_tensor("ptr_out", [BL, l_dec], fp32,
                             kind="ExternalOutput").ap()

    eproj = nc.dram_tensor("eproj", [KC, 128, BL * L], fp32, kind="Internal").ap()

    attn_flat = attn_out.rearrange("b t l -> b (t l)")

    enc_unroll = 16
    while l_enc % enc_unroll != 0:
        enc_unroll //= 2
    n_enc_outer = l_enc // enc_unroll

    with tile.TileContext(nc) as tc, ExitStack() as ctx:
        # ---------------- persistent pools ----------------
        cpool = ctx.enter_context(tc.tile_pool(name="consts", bufs=1))
        spool = ctx.enter_context(tc.tile_pool(name="state", bufs=1))
        psum = ctx.enter_context(tc.tile_pool(name="ps", bufs=1, space="PSUM"))

        xin_sb = cpool.tile([BL, L], fp32)
        nc.sync.dma_start(xin_sb[:], x_in)
        iota_sb = cpool.tile([BL, L], fp32)
        nc.sync.dma_start(iota_sb[:], iota_f)
        ident_sb = cpool.tile([BL, BL], fp32)
        nc.sync.dma_start(ident_sb[:], ident)
        vZ_sb = cpool.tile([128, KC * 127], fp32)
        nc.sync.dma_start(vZ_sb[:], vZ)
        eaug_sb = cpool.tile([2, H4], fp32)
        nc.sync.dma_start(eaug_sb[:], enc_aug)
        daug_sb = cpool.tile([2, H4], fp32)
        nc.sync.dma_start(daug_sb[:], dec_aug)
        W2T_sb = [cpool.tile([128, H], fp32, name=f"w2t{k}", tag=f"w2t{k}") for k in range(KC)]
        for k in range(KC):
            nc.sync.dma_start(W2T_sb[k][:], W2T[k * 128:(k + 1) * 128, :])
        W1b_sb = cpool.tile([128, KC], fp32)
        nc.sync.dma_start(W1b_sb[:], W1b4)
        W2b_sb = cpool.tile([128, KC], fp32)
        nc.sync.dma_start(W2b_sb[:], W2b4)
        WhhTd_sb = [cpool.tile([128, H4], fp32, name=f"whhd{k}", tag=f"whhd{k}") for k in range(KC)]
        for k in range(KC):
            nc.sync.dma_start(WhhTd_sb[k][:], WhhT_d[k * 128:(k + 1) * 128, :])

        # state: H-layout [128, (kc, b)] -> chunk k at cols k*BL
        Hs = spool.tile([128, KC * BL], fp32)
        Cs = spool.tile([128, KC * BL], fp32)
        xrow2 = spool.tile([2, BL], fp32)
        gates = spool.tile([128, MG * BL], fp32)   # tanh'd gate chunks
        tch = spool.tile([128, KC * BL], fp32)     # tanh(c/1) i.e. tanh(C*0.5)
        t1t2 = spool.tile([128, 2 * BL], fp32)     # stt temps
        qT_sb = spool.tile([128, KC * BL], fp32)   # q chunks [part, (kc, b)]
        u_sb = spool.tile([BL, L], fp32)
        max_sb = spool.tile([BL, 8], fp32)
        idx_sb = spool.tile([BL, 8], mybir.dt.uint32)
        negm_sb = spool.tile([BL, 1], fp32)
        exp_scr = spool.tile([BL, L], fp32)
        attn_sb = spool.tile([BL, L], fp32)
        mask_sb = spool.tile([BL, L], fp32)
        junk_sb = spool.tile([BL, L], fp32)
        nxt_sb = spool.tile([BL, 1], fp32)
        S_sb = spool.tile([BL, l_dec], fp32)
        ptr_sb = spool.tile([BL, l_dec], fp32)
        lz_sb = spool.tile([BL, l_dec], fp32)

        nc.vector.memset(Hs[:], 0.0)
        nc.vector.memset(Cs[:], 0.0)
        nc.vector.memset(xrow2[:], 1.0)  # row1 = ones forever; row0 rewritten per step

        # ---- shared per-step LSTM emitter (encoder & decoder) ----
        def lstm_step(WhhT_sb, aug_sb):
            gb0 = psum.tile([128, 512], fp32, tag="gb0")
            gb1 = psum.tile([128, 512], fp32, tag="gb1")
            gb = lambda mg: (gb0 if mg < 8 else gb1)[:, (mg % 8) * BL:(mg % 8) * BL + BL]
            for mg in range(MG):
                for k in range(KC):
                    nc.tensor.matmul(gb(mg), WhhT_sb[k][:, mg * 128:(mg + 1) * 128],
                                     Hs[:, k * BL:(k + 1) * BL],
                                     start=(k == 0), stop=False)
                nc.tensor.matmul(gb(mg), aug_sb[0:2, mg * 128:(mg + 1) * 128],
                                 xrow2[:], start=False, stop=True)
            # gates: i(0:4) f(4:8) in gb0, g(8:12) o(12:16) in gb1
            nc.scalar.activation(gates[:, 0:8 * BL], gb0[:], AF.Tanh, scale=0.5)
            nc.scalar.activation(gates[:, 8 * BL:12 * BL], gb1[:, 0:4 * BL],
                                 AF.Tanh, scale=1.0)
            nc.scalar.activation(gates[:, 12 * BL:16 * BL], gb1[:, 4 * BL:8 * BL],
                                 AF.Tanh, scale=0.5)
            for c in range(KC):
                ti = gates[:, c * BL:(c + 1) * BL]
                tf = gates[:, (4 + c) * BL:(5 + c) * BL]
                tg = gates[:, (8 + c) * BL:(9 + c) * BL]
                to = gates[:, (12 + c) * BL:(13 + c) * BL]
                cc = Cs[:, c * BL:(c + 1) * BL]
                T1 = t1t2[:, 0:BL]
                T2 = t1t2[:, BL:2 * BL]
                nc.vector.scalar_tensor_tensor(T1, tf, 1.0, cc, OP.add, OP.mult)
                nc.vector.scalar_tensor_tensor(T2, ti, 1.0, tg, OP.add, OP.mult)
                nc.vector.scalar_tensor_tensor(cc, T1, 0.5, T2, OP.mult, OP.add)
                nc.scalar.activation(tch[:, c * BL:(c + 1) * BL], cc, AF.Tanh,
                                     scale=0.5)
                nc.vector.scalar_tensor_tensor(Hs[:, c * BL:(c + 1) * BL], to, 1.0,
                                               tch[:, c * BL:(c + 1) * BL],
                                               OP.add, OP.mult)

        # ================= encoder =================
        with tc.tile_pool(name="enc", bufs=1) as epool, \
             tc.tile_pool(name="encst", bufs=2) as stpool:
            WhhTe_sb = [epool.tile([128, H4], fp32, name=f"whhe{k}", tag=f"whhe{k}") for k in range(KC)]
            for k in range(KC):
                nc.sync.dma_start(WhhTe_sb[k][:], WhhT_e[k * 128:(k + 1) * 128, :])
            W1T_sb = [epool.tile([128, H], fp32, name=f"w1t{k}", tag=f"w1t{k}") for k in range(KC)]
            for k in range(KC):
                nc.sync.dma_start(W1T_sb[k][:], W1T[k * 128:(k + 1) * 128, :])

            with tc.For_i(0, n_enc_outer) as oi:
                stage = [stpool.tile([128, BL * enc_unroll], fp32, name=f"stg{h}", tag=f"stg{h}")
                         for h in range(KC)]
                for ii in range(enc_unroll):
                    tt = oi * enc_unroll + ii
                    nc.sync.dma_start(xrow2[0:1, :], xT_in[ds(tt, 1), :])
                    lstm_step(WhhTe_sb, eaug_sb)
                    aq = psum.tile([128, KC * BL], fp32, tag="aq")
                    for m in range(KC):
                        for k in range(KC):
                            nc.tensor.matmul(aq[:, m * BL:(m + 1) * BL],
                                             W1T_sb[k][:, m * 128:(m + 1) * 128],
                                             Hs[:, k * BL:(k + 1) * BL],
                                             start=(k == 0), stop=(k == KC - 1))
                    # stage[m][p, b*enc_unroll + ii] = aq + W1b
                    for m in range(KC):
                        st3 = stage[m][:].rearrange("p (b s) -> p b s", s=enc_unroll)
                        nc.scalar.add(st3[:, :, ii], aq[:, m * BL:(m + 1) * BL],
                                      W1b_sb[:, m:m + 1])
                ep3 = eproj.rearrange("c p (b l) -> c p b l", b=BL)
                for m in range(KC):
                    nc.sync.dma_start(ep3[m, :, :, ds(oi * enc_unroll, enc_unroll)],
                                      stage[m][:])

        # ================= decode =================
        nc.vector.memset(xrow2[0:1, :], SOS)

        with tc.tile_pool(name="land", bufs=land_bufs) as lpool:
            with tc.For_i(0, l_dec) as t:
                lstm_step(WhhTd_sb, daug_sb)
                # qT = W2T' @ H + b
                qp = psum.tile([128, KC * BL], fp32, tag="qp")
                for m in range(KC):
                    for k in range(KC):
                        nc.tensor.matmul(qp[:, m * BL:(m + 1) * BL],
                                         W2T_sb[k][:, m * 128:(m + 1) * 128],
                                         Hs[:, k * BL:(k + 1) * BL],
                                         start=(k == 0), stop=(k == KC - 1))
                for m in range(KC):
                    nc.scalar.add(qT_sb[:, m * BL:(m + 1) * BL],
                                  qp[:, m * BL:(m + 1) * BL], W2b_sb[:, m:m + 1])

                # attention: stream eproj, add q, tanh, contract with v
                u_ps = psum.tile([BL, L], fp32, tag="ups")
                n_mm = 0
                for hc in range(KC):
                    for bg in range(nbg):
                        land = lpool.tile([128, LAND_W], fp32, tag="land")
                        nc.sync.dma_start(
                            land[:], eproj[hc, :, bg * LAND_W:(bg + 1) * LAND_W])
                        for br in range(BGRP):
                            b = bg * BGRP + br
                            nc.vector.tensor_scalar(
                                land[:, br * L:(br + 1) * L],
                                land[:, br * L:(br + 1) * L],
                                qT_sb[:, hc * BL + b:hc * BL + b + 1],
                                None, OP.add)
                        nc.scalar.activation(land[:], land[:], AF.Tanh)
                        for br in range(BGRP):
                            b = bg * BGRP + br
                            col0 = hc * 127 + 63 - b
                            n_mm += 1
                            nc.tensor.matmul(u_ps[:],
                                             vZ_sb[:, col0:col0 + BL],
                                             land[:, br * L:(br + 1) * L],
                                             start=(n_mm == 1),
                                             stop=(n_mm == KC * BL),
                                             skip_group_check=True)

                # softmax / argmax / gather
                nc.vector.tensor_copy(u_sb[:], u_ps[:])
                nc.vector.max(max_sb[:], u_sb[:])
                nc.vector.max_index(idx_sb[:], max_sb[:], u_sb[:])
                nc.vector.tensor_scalar_mul(negm_sb[:], max_sb[:, 0:1], -1.0)
                nc.scalar.activation(exp_scr[:], u_sb[:], AF.Exp,
                                     bias=negm_sb[:], scale=1.0,
                                     accum_out=S_sb[:, ds(t, 1)])
                nc.vector.tensor_scalar(attn_sb[:], u_sb[:], max_sb[:, 0:1],
                                        None, OP.subtract)
                nc.sync.dma_start(attn_flat[:, ds(t * L, L)], attn_sb[:])
                nc.vector.tensor_copy(ptr_sb[:, ds(t, 1)], idx_sb[:, 0:1])
                nc.vector.scalar_tensor_tensor(
                    junk_sb[:], iota_sb[:], ptr_sb[:, ds(t, 1)], xin_sb[:],
                    OP.is_equal, OP.mult, accum_out=nxt_sb[:])
                xr_ps = psum.tile([1, BL], fp32, tag="xrps")
                nc.tensor.transpose(xr_ps[:], nxt_sb[:], ident_sb[:])
                nc.vector.tensor_copy(xrow2[0:1, :], xr_ps[:])

        # ================= end phase: apply -log(S) =================
        nc.scalar.activation(lz_sb[:], S_sb[:], AF.Ln)
        nc.sync.dma_start(ptr_out, ptr_sb[:])
        with tc.tile_pool(name="fix", bufs=4) as fpool:
            for t in range(l_dec):
                slab = fpool.tile([BL, L], fp32, tag="slab")
                nc.sync.dma_start(slab[:], attn_flat[:, t * L:(t + 1) * L])
                nc.vector.tensor_scalar(slab[:], slab[:], lz_sb[:, t:t + 1],
                                        None, OP.subtract)
                nc.sync.dma_start(attn_flat[:, t * L:(t + 1) * L], slab[:])

    nc.compile()
    return nc


def host_constants(inputs):
    """Precompute folded/pre-scaled constants (float64 for the foldings)."""
    f8 = {k: np.asarray(v, np.float64) for k, v in inputs.items()}
    c = {}
    c["enc_aug"] = np.stack([
        f8["enc_Wih"] @ f8["enc_emb_W"][:, 0],
        f8["enc_Wih"] @ f8["enc_emb_b"] + f8["enc_bih"] + f8["enc_bhh"],
    ]).astype(np.float32)
    c["dec_aug"] = np.stack([
        f8["dec_Wih"] @ f8["dec_emb_W"][:, 0],
        f8["dec_Wih"] @ f8["dec_emb_b"] + f8["dec_bih"] + f8["dec_bhh"],
    ]).astype(np.float32)
    c["WhhT_e"] = np.ascontiguousarray((0.5 * f8["enc_Whh"]).T).astype(np.float32)
    c["WhhT_d"] = np.ascontiguousarray((0.5 * f8["dec_Whh"]).T).astype(np.float32)
    c["W1T"] = np.ascontiguousarray((0.5 * f8["W1_W"]).T).astype(np.float32)
    c["W2T"] = np.ascontiguousarray((0.5 * f8["W2_W"]).T).astype(np.float32)
    c["W1b4"] = np.ascontiguousarray(
        f8["W1_b"].astype(np.float32).reshape(KC, 128).T)
    c["W2b4"] = np.ascontiguousarray(
        f8["W2_b"].astype(np.float32).reshape(KC, 128).T)
    v = np.asarray(inputs["v_W"], np.float32)[0]
    vZ = np.zeros((128, KC * 127), np.float32)
    for hc in range(KC):
        vZ[:, hc * 127 + 63] = v[hc * 128:(hc + 1) * 128]
    c["vZ"] = vZ
    c["ident"] = np.eye(BL, dtype=np.float32)
    c["iota_f"] = np.broadcast_to(
        np.arange(L, dtype=np.float32), (BL, L)).copy()
    return c


def kernel(**inputs):
    global LAST_EXEC_NS
    import os
    from concourse.bass_utils import run_bass_kernel_spmd

    key = (L, L)
    if key not in _PROGRAM_CACHE:
        _PROGRAM_CACHE[key] = build_program(L, L)
    nc = _PROGRAM_CACHE[key]

    c = host_constants(inputs)
    x = np.asarray(inputs["input_seq"], np.float32)[:, :, 0]   # (B, L)
    in_maps = []
    for core in range(NCORES):
        sl = slice(core * BL, (core + 1) * BL)
        m = dict(c)
        m["x_in"] = np.ascontiguousarray(x[sl])
        m["xT_in"] = np.ascontiguousarray(x[sl].T)
        in_maps.append(m)

    trace = bool(int(os.environ.get("KERNEL_TRACE", "0")))
    res = run_bass_kernel_spmd(nc, in_maps, core_ids=list(range(NCORES)),
                               trace=trace)
    LAST_EXEC_NS = res.exec_time_ns if res.exec_time_ns else res.mean_exec_time_ns

    attns = np.empty((B, L, L), np.float32)
    ptrs = np.empty((B, L), np.float32)
    for core in range(NCORES):
        sl = slice(core * BL, (core + 1) * BL)
        attns[sl] = res.results[core]["attn_out"]
        ptrs[sl] = res.results[core]["ptr_out"]
    return attns, ptrs
